# revision 28
# baseline (speedup 1.0000x reference)
"""Trainium2 Bass kernel for a differentiable-DTW style module.

Math (per batch b):
    dist[i, j] = |score[b, i] - template[j]|              (i, j in [0, 2048))
    path       = softmax(-dist, axis=-1)                  (row-stochastic)
    warped[b]  = path @ feature[b]                        ([2048, 512])
    l2         = 1e-7 * sqrt(sum over all b of warped ** 2)

Implementation notes:
  - Data-parallel: batch b -> NeuronCore b (B == 8 == n_cores).
  - Softmax row sums are computed EXACTLY on the host with a sorted
    prefix-sum identity:
        sum_j exp(-|s - t_j|) = exp(-s) * sum_{t_j <= s} exp(t_j)
                              + exp(s)  * sum_{t_j >  s} exp(-t_j)
    so the device only scales matmul output rows by 1/den.
  - Kernel matrix generated directly in TRANSPOSED layout
    ET[j, i] = exp(-|s_i - t_j|) (template on partitions) == the lhsT the
    tensor engine wants; feature [j, f] is the rhs.
  - ET is bf16; generation is split between ScalarE (Abs then Exp, with a
    per-partition -t bias) and VectorE (E = min(e^s e^-t, e^-s e^t)).
  - PSUM has 8 banks but there are 16 output tiles, so matmuls run in two
    phases (output tiles 0-7, then 8-15).  Phase A only reads ET columns
    [0, 1024) and phase B only [1024, 2048), so ET is generated in
    half-width passes; pass-B halves are generated while phase-A matmuls
    run and their broadcast inputs are DMA'd late.
  - Emission order tracks intended execution order: the Tile scheduler
    batches DMA-completion semaphore ticks by schedule position, so a
    consumer emitted after an unrelated large DMA ends up waiting for it.
  - DMAs are spread over three rings (sync / scalar / gpsimd-SWDGE); each
    dma_start costs ~0.7-1.5us of issuing-engine sequencer time.
  - Output leaves the device in bf16; host upcasts and finishes the l2.
"""

import numpy as np
import ml_dtypes

B = 8
S = 2048
F = 512
P = 128
NT = S // P  # 16 chunks / output tiles
NCORES = 8
GA = 8  # phase-A output-tile group (== number of PSUM banks)
OG = 4  # output m-tiles grouped per store DMA
H = S // 2  # half of the i-range (phase A reads ET[:, :H], B the rest)
L2_REG_WEIGHT = 1e-07

# packed[:, c] column map: 0:16 -t | 16:32 1/den | 32:48 e^-t | 48:64 e^t
C_NT, C_R, C_Q, C_QI = 0, NT, 2 * NT, 3 * NT

_NC_CACHE = {}


def _build_nc():
    import concourse.mybir as mybir
    import concourse.tile as tile
    from concourse import bacc

    f32 = mybir.dt.float32
    bf16 = mybir.dt.bfloat16
    Alu = mybir.AluOpType
    Act = mybir.ActivationFunctionType

    nc = bacc.Bacc(None, target_bir_lowering=False)
    score = nc.dram_tensor("score", [S], f32, kind="ExternalInput")
    pexp = nc.dram_tensor("pexp", [S], bf16, kind="ExternalInput")
    pinv = nc.dram_tensor("pinv", [S], bf16, kind="ExternalInput")
    packed = nc.dram_tensor("packed", [P, 4 * NT], f32, kind="ExternalInput")
    ehost = nc.dram_tensor("ehost", [4, P, S], bf16, kind="ExternalInput")
    feat = nc.dram_tensor("feature", [S, F], bf16, kind="ExternalInput")
    warped = nc.dram_tensor("warped", [S, F], bf16, kind="ExternalOutput")

    # chunks 0-5 DMA'd from host; ScalarE generates odd chunks (Abs+Exp),
    # VectorE even ones via E = min(exp(s)exp(-t), exp(-s)exp(t))
    NHC = 4
    HOST_CHUNKS = tuple(range(NHC))
    ACT_CHUNKS = {5, 7, 9, 11, 13, 15}

    with tile.TileContext(nc) as tc:
        with (
            tc.tile_pool(name="const", bufs=1) as cpool,
            tc.tile_pool(name="feat", bufs=1) as fpool,
            tc.tile_pool(name="et", bufs=NT) as epool,
            tc.tile_pool(name="dtile", bufs=3) as dpool,
            tc.tile_pool(name="otile", bufs=2) as opool,
            tc.tile_pool(name="ps", bufs=8, space="PSUM") as pspool,
        ):
            ones = cpool.tile([1, P], f32, tag="ones")
            nc.gpsimd.memset(ones[:], 1.0)
            scratch = cpool.tile([1, 1], f32, tag="scratch")
            # dummy activation: forces the ACT table set load at t~0
            nc.scalar.activation(out=scratch[:], in_=ones[:, 0:1], func=Act.Exp)

            # -- startup-critical DMAs first; everything else just-in-time
            # (the scheduler lumps DMA-completion ticks by predicted order,
            # so a consumer emitted after an unrelated DMA waits for it) --
            ets = [epool.tile([P, S], bf16, tag="et", name=f"et{c}") for c in range(NT)]
            ft0 = fpool.tile([P, F], bf16, tag="ft0")
            nc.sync.dma_start(out=ft0[:], in_=feat[0:P, :])
            for c in HOST_CHUNKS:
                nc.scalar.dma_start(out=ets[c][:, 0:H], in_=ehost[c, :, 0:H])
            pk = cpool.tile([P, 4 * NT], f32, tag="pk")
            nc.sync.dma_start(out=pk[:], in_=packed[:])

            s_bcast = cpool.tile([P, S], f32, tag="sb")
            p_bc = cpool.tile([P, S], bf16, tag="pbc")
            pi_bc = cpool.tile([P, S], bf16, tag="pibc")
            nc.gpsimd.dma_start(
                out=p_bc[:, 0:H], in_=pexp[0:H].unsqueeze(0).to_broadcast([P, H])
            )
            nc.gpsimd.dma_start(
                out=pi_bc[:, 0:H], in_=pinv[0:H].unsqueeze(0).to_broadcast([P, H])
            )
            nc.scalar.dma_start(
                out=s_bcast[:, 0:H], in_=score[0:H].unsqueeze(0).to_broadcast([P, H])
            )

            ft_a = fpool.tile([P, 4 * F], bf16, tag="fta")
            ft_b1 = fpool.tile([P, 5 * F], bf16, tag="ftb1")
            ft_b2 = fpool.tile([P, 6 * F], bf16, tag="ftb2")

            def ftc(c):
                if c == 0:
                    return ft0[:]
                if c <= 4:
                    return ft_a[:, (c - 1) * F : c * F]
                if c <= 9:
                    return ft_b1[:, (c - 5) * F : (c - 4) * F]
                return ft_b2[:, (c - 10) * F : (c - 9) * F]

            def ft_dma(c):
                # just-in-time feature loads, emitted right before consumer
                if c == 1:
                    nc.sync.dma_start(
                        out=ft_a[:].rearrange("p (c f) -> p c f", c=4),
                        in_=feat[P : 5 * P, :].rearrange("(c p) f -> p c f", p=P),
                    )
                elif c == 5:
                    nc.sync.dma_start(
                        out=ft_b1[:].rearrange("p (c f) -> p c f", c=5),
                        in_=feat[5 * P : 10 * P, :].rearrange("(c p) f -> p c f", p=P),
                    )
                elif c == 10:
                    nc.sync.dma_start(
                        out=ft_b2[:].rearrange("p (c f) -> p c f", c=6),
                        in_=feat[10 * P :, :].rearrange("(c p) f -> p c f", p=P),
                    )

            def gen_half(c, h0):
                """Produce ets[c][:, h0:h0+H]."""
                sl = slice(h0, h0 + H)
                et = ets[c]
                if c in ACT_CHUNKS:
                    d = dpool.tile([P, H], f32, tag="d")
                    nc.scalar.activation(
                        out=d[:],
                        in_=s_bcast[:, sl],
                        func=Act.Abs,
                        bias=pk[:, C_NT + c : C_NT + c + 1],
                        scale=1.0,
                    )
                    nc.scalar.activation(
                        out=et[:, sl], in_=d[:], func=Act.Exp, scale=-1.0
                    )
                else:
                    m1 = dpool.tile([P, H], bf16, tag="m1")
                    m2 = dpool.tile([P, H], bf16, tag="m2")
                    nc.vector.tensor_scalar_mul(
                        m1[:], p_bc[:, sl], pk[:, C_Q + c : C_Q + c + 1]
                    )
                    nc.vector.tensor_scalar_mul(
                        m2[:], pi_bc[:, sl], pk[:, C_QI + c : C_QI + c + 1]
                    )
                    nc.vector.tensor_tensor(
                        out=et[:, sl], in0=m1[:], in1=m2[:], op=Alu.min
                    )

            ogroups = [None] * (NT // OG)

            def epilogue(ps, m):
                g, slot = divmod(m, OG)
                if ogroups[g] is None:
                    ogroups[g] = opool.tile([P, OG * F], bf16, tag="og", name=f"og{g}")
                og = ogroups[g]
                nc.vector.tensor_scalar_mul(
                    og[:, slot * F : (slot + 1) * F],
                    ps[:],
                    pk[:, C_R + m : C_R + m + 1],
                )
                if slot == OG - 1:
                    dst = warped[g * OG * P : (g + 1) * OG * P, :].rearrange(
                        "(mm p) f -> p mm f", p=P
                    )
                    nc.sync.dma_start(
                        out=dst, in_=og[:].rearrange("p (mm f) -> p mm f", mm=OG)
                    )

            # Phase A: pass-A half of each chunk, then its 8 matmuls
            psA = [
                pspool.tile([P, F], f32, tag="ps", name=f"psA{m}") for m in range(GA)
            ]
            for c in range(NT):
                ft_dma(c)
                if c not in HOST_CHUNKS:
                    gen_half(c, 0)
                for m in range(GA):
                    nc.tensor.matmul(
                        psA[m][:],
                        ets[c][:, m * P : (m + 1) * P],
                        ftc(c),
                        start=(c == 0),
                        stop=(c == NT - 1),
                    )

            # pass-B inputs (only needed from here on); the halves are
            # generated while phase-A matmuls still run
            nc.scalar.dma_start(
                out=s_bcast[:, H:S], in_=score[H:S].unsqueeze(0).to_broadcast([P, H])
            )
            nc.gpsimd.dma_start(
                out=p_bc[:, H:S], in_=pexp[H:S].unsqueeze(0).to_broadcast([P, H])
            )
            nc.gpsimd.dma_start(
                out=pi_bc[:, H:S], in_=pinv[H:S].unsqueeze(0).to_broadcast([P, H])
            )
            for c in HOST_CHUNKS:
                nc.sync.dma_start(out=ets[c][:, H:S], in_=ehost[c, :, H:S])
            for c in range(NT):
                if c not in HOST_CHUNKS:
                    gen_half(c, H)
            for m in range(GA):
                epilogue(psA[m], m)

            # Phase B: dense matmuls for output tiles 8..15
            for m in range(GA, NT):
                ps = pspool.tile([P, F], f32, tag="ps", name=f"psB{m}")
                for c in range(NT):
                    nc.tensor.matmul(
                        ps[:],
                        ets[c][:, m * P : (m + 1) * P],
                        ftc(c),
                        start=(c == 0),
                        stop=(c == NT - 1),
                    )
                epilogue(ps, m)

    nc.compile()
    return nc


def get_nc():
    if "nc" not in _NC_CACHE:
        _NC_CACHE["nc"] = _build_nc()
    return _NC_CACHE["nc"]


def _host_rden(score, template):
    """Exact softmax denominators: rden[b, i] = 1 / sum_j exp(-|s_bi - t_j|)."""
    s = score[:, :, 0].astype(np.float64)  # [B, S]
    t = np.sort(template[0, :, 0].astype(np.float64))  # [S]
    C = np.concatenate([[0.0], np.cumsum(np.exp(t))])  # C[k] = sum_{j<k} e^{t_j}
    D = np.concatenate([[0.0], np.cumsum(np.exp(-t)[::-1])])[::-1]  # sum_{j>=k} e^{-t}
    k = np.searchsorted(t, s.ravel(), side="right").reshape(s.shape)
    den = np.exp(-s) * C[k] + np.exp(s) * D[k]
    return (1.0 / den).astype(np.float32)  # [B, S]


def make_in_maps(score, feature, template):
    rden = _host_rden(score, template)
    s = np.ascontiguousarray(score[:, :, 0], dtype=np.float32)  # [B, S]
    s64 = s.astype(np.float64)
    t64 = template[0, :, 0].astype(np.float64)  # [S]
    bf = ml_dtypes.bfloat16

    def colmaj(v):  # [S] -> [128, 16] with v[c*128+p] at [p, c]
        return np.asarray(v, dtype=np.float32).reshape(NT, P).T

    nt_cols = colmaj(-t64)
    q_cols = colmaj(np.exp(-t64))
    qi_cols = colmaj(np.exp(t64))
    # host-computed kernel-matrix chunks 0..3: [B, 4, 128, S] bf16
    t_hc = t64[: 4 * P]
    eh_all = (
        np.exp(-np.abs(s64[:, None, :] - t_hc[None, :, None]))
        .astype(bf)
        .reshape(B, 4, P, S)
    )
    in_maps = []
    for b in range(B):
        pk = np.concatenate(
            [nt_cols, colmaj(rden[b]), q_cols, qi_cols], axis=1
        )  # [128, 64]
        in_maps.append(
            {
                "score": s[b],
                "pexp": np.exp(s64[b]).astype(bf),
                "pinv": np.exp(-s64[b]).astype(bf),
                "packed": np.ascontiguousarray(pk),
                "ehost": eh_all[b],
                "feature": np.asarray(feature[b], dtype=np.float32).astype(bf),
            }
        )
    return in_maps


def postprocess(results):
    """results: per-core list of {name: np.ndarray} -> (warped, l2)."""
    warped = np.stack(
        [np.asarray(results[b]["warped"]).astype(np.float32) for b in range(B)]
    )
    l2 = np.float32(L2_REG_WEIGHT * np.sqrt(np.sum(warped.astype(np.float64) ** 2)))
    return warped, l2


def kernel(score, feature, template):
    from concourse.bass_utils import run_bass_kernel_spmd

    nc = get_nc()
    in_maps = make_in_maps(score, feature, template)
    res = run_bass_kernel_spmd(nc, in_maps, core_ids=list(range(NCORES)))
    return postprocess(res.results)


# revision 29
# speedup vs baseline: 1.0569x; 1.0569x over previous
"""Trainium2 Bass kernel for a differentiable-DTW style module.

Math (per batch b):
    dist[i, j] = |score[b, i] - template[j]|              (i, j in [0, 2048))
    path       = softmax(-dist, axis=-1)                  (row-stochastic)
    warped[b]  = path @ feature[b]                        ([2048, 512])
    l2         = 1e-7 * sqrt(sum over all b of warped ** 2)

Implementation notes:
  - Data-parallel: batch b -> NeuronCore b (B == 8 == n_cores).
  - Softmax row sums are computed EXACTLY on the host with a sorted
    prefix-sum identity:
        sum_j exp(-|s - t_j|) = exp(-s) * sum_{t_j <= s} exp(t_j)
                              + exp(s)  * sum_{t_j >  s} exp(-t_j)
    so the device only scales matmul output rows by 1/den.
  - Kernel matrix generated directly in TRANSPOSED layout
    ET[j, i] = exp(-|s_i - t_j|) (template on partitions) == the lhsT the
    tensor engine wants; feature [j, f] is the rhs.
  - ET is bf16; generation is split between ScalarE (Abs then Exp, with a
    per-partition -t bias) and VectorE (E = min(e^s e^-t, e^-s e^t)).
  - PSUM has 8 banks but there are 16 output tiles, so matmuls run in two
    phases (output tiles 0-7, then 8-15).  Phase A only reads ET columns
    [0, 1024) and phase B only [1024, 2048), so ET is generated in
    half-width passes; pass-B halves are generated while phase-A matmuls
    run and their broadcast inputs are DMA'd late.
  - Emission order tracks intended execution order: the Tile scheduler
    batches DMA-completion semaphore ticks by schedule position, so a
    consumer emitted after an unrelated large DMA ends up waiting for it.
  - DMAs are spread over three rings (sync / scalar / gpsimd-SWDGE); each
    dma_start costs ~0.7-1.5us of issuing-engine sequencer time.
  - Output leaves the device in bf16; host upcasts and finishes the l2.
"""

import numpy as np
import ml_dtypes

B = 8
S = 2048
F = 512
P = 128
NT = S // P  # 16 chunks / output tiles
NCORES = 8
GA = 8  # phase-A output-tile group (== number of PSUM banks)
OG = 4  # output m-tiles grouped per store DMA
H = S // 2  # half of the i-range (phase A reads ET[:, :H], B the rest)
L2_REG_WEIGHT = 1e-07

# packed[:, c] column map: 0:16 -t | 16:32 1/den | 32:48 e^-t | 48:64 e^t
C_NT, C_R, C_Q, C_QI = 0, NT, 2 * NT, 3 * NT

_NC_CACHE = {}


def _build_nc():
    import concourse.mybir as mybir
    import concourse.tile as tile
    from concourse import bacc

    f32 = mybir.dt.float32
    bf16 = mybir.dt.bfloat16
    Alu = mybir.AluOpType
    Act = mybir.ActivationFunctionType

    nc = bacc.Bacc(None, target_bir_lowering=False)
    score = nc.dram_tensor("score", [S], f32, kind="ExternalInput")
    pexp = nc.dram_tensor("pexp", [S], bf16, kind="ExternalInput")
    pinv = nc.dram_tensor("pinv", [S], bf16, kind="ExternalInput")
    packed = nc.dram_tensor("packed", [P, 4 * NT], f32, kind="ExternalInput")
    feat = nc.dram_tensor("feature", [S, F], bf16, kind="ExternalInput")
    warped = nc.dram_tensor("warped", [S, F], bf16, kind="ExternalOutput")

    # ScalarE-generated chunks (Abs+Exp); the rest on VectorE via
    # E = min(exp(s)exp(-t), exp(-s)exp(t))  (exactly exp(-|s-t|))
    ACT_CHUNKS = {0, 2, 4, 6, 8, 10}

    with tile.TileContext(nc) as tc:
        with (
            tc.tile_pool(name="const", bufs=1) as cpool,
            tc.tile_pool(name="feat", bufs=1) as fpool,
            tc.tile_pool(name="et", bufs=NT) as epool,
            tc.tile_pool(name="dtile", bufs=3) as dpool,
            tc.tile_pool(name="otile", bufs=2) as opool,
            tc.tile_pool(name="ps", bufs=8, space="PSUM") as pspool,
        ):
            ones = cpool.tile([1, P], f32, tag="ones")
            nc.gpsimd.memset(ones[:], 1.0)
            scratch = cpool.tile([1, 1], f32, tag="scratch")
            # dummy activation: forces the ACT table set load at t~0
            nc.scalar.activation(out=scratch[:], in_=ones[:, 0:1], func=Act.Exp)

            # -- DMA waves: pk + ft0 first, warmup matmuls right behind
            # them, then the pass-A broadcast halves, then bulk features --
            ets = [epool.tile([P, S], bf16, tag="et", name=f"et{c}") for c in range(NT)]
            pk = cpool.tile([P, 4 * NT], f32, tag="pk")
            nc.sync.dma_start(out=pk[:], in_=packed[:])
            ft0 = fpool.tile([P, F], bf16, tag="ft0")
            nc.sync.dma_start(out=ft0[:], in_=feat[0:P, :])

            wps = pspool.tile([P, F], f32, tag="ps", name="warmup_ps")
            for _ in range(8):
                nc.tensor.matmul(wps[:], ft0[:, 0:P], ft0[:], start=True, stop=True)

            s_bcast = cpool.tile([P, S], f32, tag="sb")
            p_bc = cpool.tile([P, S], bf16, tag="pbc")
            pi_bc = cpool.tile([P, S], bf16, tag="pibc")
            for h0 in (0, H):
                sl = slice(h0, h0 + H)
                nc.scalar.dma_start(
                    out=s_bcast[:, sl],
                    in_=score[sl].unsqueeze(0).to_broadcast([P, H]),
                )
                nc.gpsimd.dma_start(
                    out=p_bc[:, sl], in_=pexp[sl].unsqueeze(0).to_broadcast([P, H])
                )
                nc.gpsimd.dma_start(
                    out=pi_bc[:, sl], in_=pinv[sl].unsqueeze(0).to_broadcast([P, H])
                )

            ft_a = fpool.tile([P, 4 * F], bf16, tag="fta")
            nc.sync.dma_start(
                out=ft_a[:].rearrange("p (c f) -> p c f", c=4),
                in_=feat[P : 5 * P, :].rearrange("(c p) f -> p c f", p=P),
            )
            ft_b = fpool.tile([P, (NT - 5) * F], bf16, tag="ftb")
            nc.sync.dma_start(
                out=ft_b[:].rearrange("p (c f) -> p c f", c=NT - 5),
                in_=feat[5 * P :, :].rearrange("(c p) f -> p c f", p=P),
            )

            def ftc(c):
                if c == 0:
                    return ft0[:]
                if c <= 4:
                    return ft_a[:, (c - 1) * F : c * F]
                return ft_b[:, (c - 5) * F : (c - 4) * F]

            def gen_half(c, h0):
                """Produce ets[c][:, h0:h0+H]."""
                sl = slice(h0, h0 + H)
                et = ets[c]
                if c in ACT_CHUNKS:
                    d = dpool.tile([P, H], f32, tag="d")
                    nc.scalar.activation(
                        out=d[:],
                        in_=s_bcast[:, sl],
                        func=Act.Abs,
                        bias=pk[:, C_NT + c : C_NT + c + 1],
                        scale=1.0,
                    )
                    nc.scalar.activation(
                        out=et[:, sl], in_=d[:], func=Act.Exp, scale=-1.0
                    )
                else:
                    m1 = dpool.tile([P, H], bf16, tag="m1")
                    m2 = dpool.tile([P, H], bf16, tag="m2")
                    nc.vector.tensor_scalar_mul(
                        m1[:], p_bc[:, sl], pk[:, C_Q + c : C_Q + c + 1]
                    )
                    nc.vector.tensor_scalar_mul(
                        m2[:], pi_bc[:, sl], pk[:, C_QI + c : C_QI + c + 1]
                    )
                    nc.vector.tensor_tensor(
                        out=et[:, sl], in0=m1[:], in1=m2[:], op=Alu.min
                    )

            ogroups = [None] * (NT // OG)

            def epilogue(ps, m):
                g, slot = divmod(m, OG)
                if ogroups[g] is None:
                    ogroups[g] = opool.tile([P, OG * F], bf16, tag="og", name=f"og{g}")
                og = ogroups[g]
                nc.vector.tensor_scalar_mul(
                    og[:, slot * F : (slot + 1) * F],
                    ps[:],
                    pk[:, C_R + m : C_R + m + 1],
                )
                if slot == OG - 1:
                    dst = warped[g * OG * P : (g + 1) * OG * P, :].rearrange(
                        "(mm p) f -> p mm f", p=P
                    )
                    nc.sync.dma_start(
                        out=dst, in_=og[:].rearrange("p (mm f) -> p mm f", mm=OG)
                    )

            # Phase A: pass-A half of each chunk, then its 8 matmuls
            psA = [
                pspool.tile([P, F], f32, tag="ps", name=f"psA{m}") for m in range(GA)
            ]
            for c in range(NT):
                gen_half(c, 0)
                for m in range(GA):
                    nc.tensor.matmul(
                        psA[m][:],
                        ets[c][:, m * P : (m + 1) * P],
                        ftc(c),
                        start=(c == 0),
                        stop=(c == NT - 1),
                    )

            # pass-B halves: generated while phase-A matmuls still run
            for c in range(NT):
                gen_half(c, H)
            for m in range(GA):
                epilogue(psA[m], m)

            # Phase B: dense matmuls for output tiles 8..15
            for m in range(GA, NT):
                ps = pspool.tile([P, F], f32, tag="ps", name=f"psB{m}")
                for c in range(NT):
                    nc.tensor.matmul(
                        ps[:],
                        ets[c][:, m * P : (m + 1) * P],
                        ftc(c),
                        start=(c == 0),
                        stop=(c == NT - 1),
                    )
                epilogue(ps, m)

    nc.compile()
    return nc


def get_nc():
    if "nc" not in _NC_CACHE:
        _NC_CACHE["nc"] = _build_nc()
    return _NC_CACHE["nc"]


def _host_rden(score, template):
    """Exact softmax denominators: rden[b, i] = 1 / sum_j exp(-|s_bi - t_j|)."""
    s = score[:, :, 0].astype(np.float64)  # [B, S]
    t = np.sort(template[0, :, 0].astype(np.float64))  # [S]
    C = np.concatenate([[0.0], np.cumsum(np.exp(t))])  # C[k] = sum_{j<k} e^{t_j}
    D = np.concatenate([[0.0], np.cumsum(np.exp(-t)[::-1])])[::-1]  # sum_{j>=k} e^{-t}
    k = np.searchsorted(t, s.ravel(), side="right").reshape(s.shape)
    den = np.exp(-s) * C[k] + np.exp(s) * D[k]
    return (1.0 / den).astype(np.float32)  # [B, S]


def make_in_maps(score, feature, template):
    rden = _host_rden(score, template)
    s = np.ascontiguousarray(score[:, :, 0], dtype=np.float32)  # [B, S]
    s64 = s.astype(np.float64)
    t64 = template[0, :, 0].astype(np.float64)  # [S]
    bf = ml_dtypes.bfloat16

    def colmaj(v):  # [S] -> [128, 16] with v[c*128+p] at [p, c]
        return np.asarray(v, dtype=np.float32).reshape(NT, P).T

    nt_cols = colmaj(-t64)
    q_cols = colmaj(np.exp(-t64))
    qi_cols = colmaj(np.exp(t64))
    in_maps = []
    for b in range(B):
        pk = np.concatenate(
            [nt_cols, colmaj(rden[b]), q_cols, qi_cols], axis=1
        )  # [128, 64]
        in_maps.append(
            {
                "score": s[b],
                "pexp": np.exp(s64[b]).astype(bf),
                "pinv": np.exp(-s64[b]).astype(bf),
                "packed": np.ascontiguousarray(pk),
                "feature": np.asarray(feature[b], dtype=np.float32).astype(bf),
            }
        )
    return in_maps


def postprocess(results):
    """results: per-core list of {name: np.ndarray} -> (warped, l2)."""
    warped = np.stack(
        [np.asarray(results[b]["warped"]).astype(np.float32) for b in range(B)]
    )
    l2 = np.float32(L2_REG_WEIGHT * np.sqrt(np.sum(warped.astype(np.float64) ** 2)))
    return warped, l2


def kernel(score, feature, template):
    from concourse.bass_utils import run_bass_kernel_spmd

    nc = get_nc()
    in_maps = make_in_maps(score, feature, template)
    res = run_bass_kernel_spmd(nc, in_maps, core_ids=list(range(NCORES)))
    return postprocess(res.results)


# revision 30
# speedup vs baseline: 1.0784x; 1.0204x over previous
"""Trainium2 Bass kernel for a differentiable-DTW style module.

Math (per batch b):
    dist[i, j] = |score[b, i] - template[j]|              (i, j in [0, 2048))
    path       = softmax(-dist, axis=-1)                  (row-stochastic)
    warped[b]  = path @ feature[b]                        ([2048, 512])
    l2         = 1e-7 * sqrt(sum over all b of warped ** 2)

Implementation notes:
  - Data-parallel: batch b -> NeuronCore b (B == 8 == n_cores).
  - Softmax row sums are computed EXACTLY on the host with a sorted
    prefix-sum identity:
        sum_j exp(-|s - t_j|) = exp(-s) * sum_{t_j <= s} exp(t_j)
                              + exp(s)  * sum_{t_j >  s} exp(-t_j)
    so the device only scales matmul output rows by 1/den.
  - Kernel matrix generated directly in TRANSPOSED layout
    ET[j, i] = exp(-|s_i - t_j|) (template on partitions) == the lhsT the
    tensor engine wants; feature [j, f] is the rhs.
  - ET is bf16; generation is split between ScalarE (Abs then Exp, with a
    per-partition -t bias) and VectorE (E = min(e^s e^-t, e^-s e^t)).
  - PSUM has 8 banks but there are 16 output tiles, so matmuls run in two
    phases (output tiles 0-7, then 8-15).  Phase A only reads ET columns
    [0, 1024) and phase B only [1024, 2048), so ET is generated in
    half-width passes; pass-B halves are generated while phase-A matmuls
    run and their broadcast inputs are DMA'd late.
  - Emission order tracks intended execution order: the Tile scheduler
    batches DMA-completion semaphore ticks by schedule position, so a
    consumer emitted after an unrelated large DMA ends up waiting for it.
  - DMAs are spread over three rings (sync / scalar / gpsimd-SWDGE); each
    dma_start costs ~0.7-1.5us of issuing-engine sequencer time.
  - Output leaves the device in bf16; host upcasts and finishes the l2.
"""

import numpy as np
import ml_dtypes

B = 8
S = 2048
F = 512
P = 128
NT = S // P  # 16 chunks / output tiles
NCORES = 8
GA = 8  # phase-A output-tile group (== number of PSUM banks)
OG = 4  # output m-tiles grouped per store DMA
H = S // 2  # half of the i-range (phase A reads ET[:, :H], B the rest)
L2_REG_WEIGHT = 1e-07

# packed[:, c] column map: 0:16 -t | 16:32 1/den | 32:48 e^-t | 48:64 e^t
C_NT, C_R, C_Q, C_QI = 0, NT, 2 * NT, 3 * NT

_NC_CACHE = {}


def _build_nc():
    import concourse.mybir as mybir
    import concourse.tile as tile
    from concourse import bacc

    f32 = mybir.dt.float32
    bf16 = mybir.dt.bfloat16
    Alu = mybir.AluOpType
    Act = mybir.ActivationFunctionType

    nc = bacc.Bacc(None, target_bir_lowering=False)
    score = nc.dram_tensor("score", [S], f32, kind="ExternalInput")
    pexp = nc.dram_tensor("pexp", [S], bf16, kind="ExternalInput")
    pinv = nc.dram_tensor("pinv", [S], bf16, kind="ExternalInput")
    packed = nc.dram_tensor("packed", [P, 4 * NT], f32, kind="ExternalInput")
    feat = nc.dram_tensor("feature", [S, F], bf16, kind="ExternalInput")
    warped = nc.dram_tensor("warped", [S, F], bf16, kind="ExternalOutput")

    # ScalarE-generated chunks (Abs+Exp); the rest on VectorE via
    # E = min(exp(s)exp(-t), exp(-s)exp(t))  (exactly exp(-|s-t|))
    ACT_CHUNKS = {0, 2, 4, 6, 8, 10}

    with tile.TileContext(nc) as tc:
        with (
            tc.tile_pool(name="const", bufs=1) as cpool,
            tc.tile_pool(name="feat", bufs=1) as fpool,
            tc.tile_pool(name="et", bufs=NT) as epool,
            tc.tile_pool(name="dtile", bufs=3) as dpool,
            tc.tile_pool(name="otile", bufs=2) as opool,
            tc.tile_pool(name="ps", bufs=8, space="PSUM") as pspool,
        ):
            ones = cpool.tile([1, P], f32, tag="ones")
            nc.gpsimd.memset(ones[:], 1.0)
            scratch = cpool.tile([1, 1], f32, tag="scratch")
            # dummy activation: forces the ACT table set load at t~0
            nc.scalar.activation(out=scratch[:], in_=ones[:, 0:1], func=Act.Exp)

            # -- DMA waves: pk + ft0 first, warmup matmuls right behind
            # them, then the pass-A broadcast halves, then bulk features --
            ets = [epool.tile([P, S], bf16, tag="et", name=f"et{c}") for c in range(NT)]
            pk = cpool.tile([P, 4 * NT], f32, tag="pk")
            nc.sync.dma_start(out=pk[:], in_=packed[:])
            ft0 = fpool.tile([P, F], bf16, tag="ft0")
            nc.sync.dma_start(out=ft0[:], in_=feat[0:P, :])

            wps = pspool.tile([P, F], f32, tag="ps", name="warmup_ps")
            for _ in range(8):
                nc.tensor.matmul(wps[:], ft0[:, 0:P], ft0[:], start=True, stop=True)

            s_bcast = cpool.tile([P, S], f32, tag="sb")
            p_bc = cpool.tile([P, S], bf16, tag="pbc")
            pi_bc = cpool.tile([P, S], bf16, tag="pibc")
            for h0 in (0, H):
                sl = slice(h0, h0 + H)
                nc.scalar.dma_start(
                    out=s_bcast[:, sl],
                    in_=score[sl].unsqueeze(0).to_broadcast([P, H]),
                )
                nc.gpsimd.dma_start(
                    out=p_bc[:, sl], in_=pexp[sl].unsqueeze(0).to_broadcast([P, H])
                )
                nc.gpsimd.dma_start(
                    out=pi_bc[:, sl], in_=pinv[sl].unsqueeze(0).to_broadcast([P, H])
                )

            ft_a = fpool.tile([P, 4 * F], bf16, tag="fta")
            nc.sync.dma_start(
                out=ft_a[:].rearrange("p (c f) -> p c f", c=4),
                in_=feat[P : 5 * P, :].rearrange("(c p) f -> p c f", p=P),
            )
            ft_b = fpool.tile([P, (NT - 5) * F], bf16, tag="ftb")
            nc.sync.dma_start(
                out=ft_b[:].rearrange("p (c f) -> p c f", c=NT - 5),
                in_=feat[5 * P :, :].rearrange("(c p) f -> p c f", p=P),
            )

            def ftc(c):
                if c == 0:
                    return ft0[:]
                if c <= 4:
                    return ft_a[:, (c - 1) * F : c * F]
                return ft_b[:, (c - 5) * F : (c - 4) * F]

            def gen_half(c, h0):
                """Produce ets[c][:, h0:h0+H]."""
                sl = slice(h0, h0 + H)
                et = ets[c]
                if c in ACT_CHUNKS:
                    d = dpool.tile([P, H], f32, tag="d")
                    nc.scalar.activation(
                        out=d[:],
                        in_=s_bcast[:, sl],
                        func=Act.Abs,
                        bias=pk[:, C_NT + c : C_NT + c + 1],
                        scale=1.0,
                    )
                    nc.scalar.activation(
                        out=et[:, sl], in_=d[:], func=Act.Exp, scale=-1.0
                    )
                else:
                    m1 = dpool.tile([P, H], bf16, tag="m1")
                    m2 = dpool.tile([P, H], bf16, tag="m2")
                    nc.vector.tensor_scalar_mul(
                        m1[:], p_bc[:, sl], pk[:, C_Q + c : C_Q + c + 1]
                    )
                    nc.vector.tensor_scalar_mul(
                        m2[:], pi_bc[:, sl], pk[:, C_QI + c : C_QI + c + 1]
                    )
                    nc.vector.tensor_tensor(
                        out=et[:, sl], in0=m1[:], in1=m2[:], op=Alu.min
                    )

            ogroups = [None] * (NT // OG)

            def epilogue(ps, m):
                g, slot = divmod(m, OG)
                if ogroups[g] is None:
                    ogroups[g] = opool.tile([P, OG * F], bf16, tag="og", name=f"og{g}")
                og = ogroups[g]
                nc.vector.tensor_scalar_mul(
                    og[:, slot * F : (slot + 1) * F],
                    ps[:],
                    pk[:, C_R + m : C_R + m + 1],
                )

                def store(lo, hi):  # m-tiles [g*OG+lo, g*OG+hi) of this group
                    dst = warped[(g * OG + lo) * P : (g * OG + hi) * P, :].rearrange(
                        "(mm p) f -> p mm f", p=P
                    )
                    nc.sync.dma_start(
                        out=dst,
                        in_=og[:, lo * F : hi * F].rearrange(
                            "p (mm f) -> p mm f", mm=hi - lo
                        ),
                    )

                if g == NT // OG - 1:
                    # last group: store in halves so only 512KB trails the
                    # final matmul
                    if slot == 1:
                        store(0, 2)
                    elif slot == OG - 1:
                        store(2, OG)
                elif slot == OG - 1:
                    store(0, OG)

            # Phase A: pass-A half of each chunk, then its 8 matmuls
            psA = [
                pspool.tile([P, F], f32, tag="ps", name=f"psA{m}") for m in range(GA)
            ]
            for c in range(NT):
                gen_half(c, 0)
                for m in range(GA):
                    nc.tensor.matmul(
                        psA[m][:],
                        ets[c][:, m * P : (m + 1) * P],
                        ftc(c),
                        start=(c == 0),
                        stop=(c == NT - 1),
                    )

            # pass-B halves: generated while phase-A matmuls still run
            for c in range(NT):
                gen_half(c, H)
            for m in range(GA):
                epilogue(psA[m], m)

            # Phase B: dense matmuls for output tiles 8..15
            for m in range(GA, NT):
                ps = pspool.tile([P, F], f32, tag="ps", name=f"psB{m}")
                for c in range(NT):
                    nc.tensor.matmul(
                        ps[:],
                        ets[c][:, m * P : (m + 1) * P],
                        ftc(c),
                        start=(c == 0),
                        stop=(c == NT - 1),
                    )
                epilogue(ps, m)

    nc.compile()
    return nc


def get_nc():
    if "nc" not in _NC_CACHE:
        _NC_CACHE["nc"] = _build_nc()
    return _NC_CACHE["nc"]


def _host_rden(score, template):
    """Exact softmax denominators: rden[b, i] = 1 / sum_j exp(-|s_bi - t_j|)."""
    s = score[:, :, 0].astype(np.float64)  # [B, S]
    t = np.sort(template[0, :, 0].astype(np.float64))  # [S]
    C = np.concatenate([[0.0], np.cumsum(np.exp(t))])  # C[k] = sum_{j<k} e^{t_j}
    D = np.concatenate([[0.0], np.cumsum(np.exp(-t)[::-1])])[::-1]  # sum_{j>=k} e^{-t}
    k = np.searchsorted(t, s.ravel(), side="right").reshape(s.shape)
    den = np.exp(-s) * C[k] + np.exp(s) * D[k]
    return (1.0 / den).astype(np.float32)  # [B, S]


def make_in_maps(score, feature, template):
    rden = _host_rden(score, template)
    s = np.ascontiguousarray(score[:, :, 0], dtype=np.float32)  # [B, S]
    s64 = s.astype(np.float64)
    t64 = template[0, :, 0].astype(np.float64)  # [S]
    bf = ml_dtypes.bfloat16

    def colmaj(v):  # [S] -> [128, 16] with v[c*128+p] at [p, c]
        return np.asarray(v, dtype=np.float32).reshape(NT, P).T

    nt_cols = colmaj(-t64)
    q_cols = colmaj(np.exp(-t64))
    qi_cols = colmaj(np.exp(t64))
    in_maps = []
    for b in range(B):
        pk = np.concatenate(
            [nt_cols, colmaj(rden[b]), q_cols, qi_cols], axis=1
        )  # [128, 64]
        in_maps.append(
            {
                "score": s[b],
                "pexp": np.exp(s64[b]).astype(bf),
                "pinv": np.exp(-s64[b]).astype(bf),
                "packed": np.ascontiguousarray(pk),
                "feature": np.asarray(feature[b], dtype=np.float32).astype(bf),
            }
        )
    return in_maps


def postprocess(results):
    """results: per-core list of {name: np.ndarray} -> (warped, l2)."""
    warped = np.stack(
        [np.asarray(results[b]["warped"]).astype(np.float32) for b in range(B)]
    )
    l2 = np.float32(L2_REG_WEIGHT * np.sqrt(np.sum(warped.astype(np.float64) ** 2)))
    return warped, l2


def kernel(score, feature, template):
    from concourse.bass_utils import run_bass_kernel_spmd

    nc = get_nc()
    in_maps = make_in_maps(score, feature, template)
    res = run_bass_kernel_spmd(nc, in_maps, core_ids=list(range(NCORES)))
    return postprocess(res.results)
